# revision 19
# baseline (speedup 1.0000x reference)
"""Trainium2 Bass kernel for DirectionalFreqEmbed (per-token gather + grouped GEMM).

Token-parallel across 8 NeuronCores. x is kept SBUF-resident in a dual-
orientation layout ([128, 24576] bf16: partitions 0-63 = batches in row-major
spatial order, 64-127 = batches in column-major order; rows interleave the 3
channel-group planes so each token's ragged index set collapses to ~4 long
contiguous runs). The per-token W stream is compacted to the actually-used
l-slots (~5.3 chunks of 128 instead of 12): holes and chunk tails carry zero
W rows so gathered garbage cannot contribute, and the bias is folded in as a
reserved slot-0 row. Each 128-chunk is transposed to [l, batch] with a single
PE matmul against a stacked-identity selector (which also masks the unused
orientation half), then accumulated against the streamed W tile.

kernel(**inputs) takes FULL unsharded inputs and returns the FULL output.
"""
import os
import sys

import ml_dtypes
import numpy as np

for _p in ("/opt/trn_rl_repo", "/root/.axon_site/_ro/trn_rl_repo"):
    if os.path.isdir(_p) and _p not in sys.path:
        sys.path.insert(0, _p)

try:  # the staged antenv lacks axon_hooks; inject a functional stand-in
    import antenv.axon_hooks  # noqa: F401
except ImportError:
    import types as _types

    _hooks = _types.ModuleType("antenv.axon_hooks")
    _hooks._hook = None
    _hooks.get_axon_ntff_profile_hook = lambda: _hooks._hook
    _hooks.set_axon_ntff_profile_hook = lambda h: setattr(_hooks, "_hook", h)
    sys.modules["antenv.axon_hooks"] = _hooks

import jax
import concourse.bass as bass  # noqa: F401
import concourse.tile as tile
from concourse import bacc, mybir

IMG, CIN, DIM, B = 64, 30, 384, 64
T, Lmax = 240, 1452
SLAB = 3 * IMG * IMG          # 12288 interleaved rows per 3-channel slab
MAXCH = 13                    # max 128-chunks per token (Lmax token)
GTILE = MAXCH * 128           # g tile free width
WGROUP = 32                   # max chunks per W DMA group
YGRP = 6                      # tokens per output tile

bf16 = mybir.dt.bfloat16
f32 = mybir.dt.float32

_cache = {}


# ---------------------------------------------------------------- host plan --

def _runs_from_sorted(rows, orig):
    """Maximal +1-contiguous runs over sorted rows; equal rows split runs."""
    runs = []
    i, n = 0, len(rows)
    while i < n:
        j = i
        while j + 1 < n and rows[j + 1] == rows[j] + 1:
            j += 1
        runs.append((int(rows[i]), int(j - i + 1), orig[i:j + 1]))
        i = j + 1
    return runs


def _merge_runs(runs, gap):
    """Merge runs whose src gap is in [0, gap]; holes become zero-W slots."""
    if not runs:
        return []
    runs = sorted(runs, key=lambda r: r[0])
    out = [list(runs[0])]
    for s, ln, orig in runs[1:]:
        g = s - (out[-1][0] + out[-1][1])
        if 0 <= g <= gap:
            out[-1][1] = s + ln - out[-1][0]
            out[-1][2] = np.concatenate([out[-1][2], orig])
        else:
            out.append([s, ln, orig])
    return [(s, ln, orig) for s, ln, orig in out]


def _token_candidate(rn, rt, minrun, gap):
    """Split elements between orientations, build merged runs per orientation."""
    L = len(rn)
    idx = np.arange(L)
    if minrun is None:        # all normal
        o = np.argsort(rn, kind="stable")
        runs_n = _runs_from_sorted(rn[o], o)
        runs_t = []
    elif minrun == 0:         # all transposed
        o = np.argsort(rt, kind="stable")
        runs_t = _runs_from_sorted(rt[o], o)
        runs_n = []
    else:
        o = np.argsort(rn, kind="stable")
        raw = _runs_from_sorted(rn[o], o)
        runs_n, lo = [], []
        for s, ln, orig in raw:
            if ln >= minrun:
                runs_n.append((s, ln, orig))
            else:
                lo.append(orig)
        runs_t = []
        if lo:
            lo = np.concatenate(lo)
            o2 = lo[np.argsort(rt[lo], kind="stable")]
            runs_t = _runs_from_sorted(rt[o2], o2)
    mn = _merge_runs(runs_n, gap)
    mt = _merge_runs(runs_t, gap)
    assert sum(r[1] >= 1 for r in mn + mt) == len(mn + mt)
    assert sum(len(r[2]) for r in mn + mt) == L
    return mn, mt


def _plan_token(t, s_bit, ia, ib, ic, lens):
    """Best gather/pack plan for one token.

    Returns dict: copies [(part, src_col, len, dst_slot)], orients per chunk,
    nch, wrows [(slot, elem)] mapping l-slot -> original W row."""
    L = int(lens[t])
    a = ia[t, :L].astype(np.int64)
    b = ib[t, :L].astype(np.int64)
    g3 = ic[t, :L].astype(np.int64) // 10
    rn = (a * 64 + b) * 3 + g3
    rt = (b * 64 + a) * 3 + g3

    best = None
    for minrun in (None, 0, 24, 48, 96):
        for gap in (16, 48):
            mn, mt = _token_candidate(rn, rt, minrun, gap)
            slots_n = sum(r[1] for r in mn)
            slots_t = sum(r[1] for r in mt)
            nch = sum(-(-s // 128) for s in (slots_n, slots_t) if s)
            ncp = len(mn) + len(mt)
            cost = ncp + 4.0 * nch
            if best is None or cost < best[0]:
                best = (cost, mn, mt, nch)
    _, mn, mt, nch = best

    copies, wrows, orients = [], [], []
    base = s_bit * SLAB
    pos = 0
    for part, merged, rows in ((0, mn, rn), (64, mt, rt)):
        if not merged:
            continue
        pos = -(-pos // 128) * 128    # each section starts on chunk boundary
        sec_start_ch = pos // 128
        for s, ln, orig in merged:
            copies.append((part, base + s, ln, pos))
            r = rows[orig]
            for e, slot in zip(orig, pos + (r - s)):
                wrows.append((int(slot), int(e)))
            pos += ln
        sec_end_ch = -(-pos // 128)
        orients.extend([0 if part == 0 else 1] * (sec_end_ch - sec_start_ch))
        pos = sec_end_ch * 128
    assert len(orients) == nch <= MAXCH
    return {"tid": t, "copies": copies, "orients": orients, "nch": nch,
            "wrows": wrows}


def _assign_cores(ia, ib, ic, lens):
    """Core k gets family k (24 tokens) + a chunk-balanced slice of family
    8 (k<4) or 9 (k>=4). Returns per-core ordered token plans."""
    cg = (np.asarray(ic)[:, 0] % 10).astype(int)
    fam = [list(np.where(cg == f)[0]) for f in range(10)]

    plans = {}
    for t in range(T):
        plans[t] = None  # filled lazily with proper s_bit per core

    def tok_plan(t, s_bit):
        return _plan_token(t, s_bit, ia, ib, ic, lens)

    # chunk counts (s_bit irrelevant for nch)
    nch = {}
    for t in range(T):
        p = tok_plan(t, 0)
        nch[t] = p["nch"]

    cores = []
    for k in range(8):
        cores.append(list(fam[k]))
    for r, ks in ((8, [0, 1, 2, 3]), (9, [4, 5, 6, 7])):
        toks = sorted(fam[r], key=lambda t: -nch[t])
        load = {k: sum(nch[t] for t in cores[k]) for k in ks}
        cnt = {k: 0 for k in ks}
        for t in toks:
            k = min([kk for kk in ks if cnt[kk] < 6], key=lambda kk: load[kk])
            cores[k].append(t)
            load[k] += nch[t]
            cnt[k] += 1

    out = []
    for k in range(8):
        fa = k
        toks = cores[k]
        # pair tokens with similar chunk counts so interleaved main matmuls
        # (PE column-group pairing) rarely run solo; famA stays first (its x
        # slab arrives first)
        fam_a = sorted(toks[:24], key=lambda t: -nch[t])
        fam_b = sorted(toks[24:], key=lambda t: -nch[t])
        toks = fam_a + fam_b
        tplans = []
        for t in toks:
            s_bit = 0 if cg[t] == fa else 1
            tplans.append(tok_plan(t, s_bit))
        out.append(tplans)
    return out, cg


def _build_inputs(x, W, bias, core_tplans, cg):
    """Per-core numpy inputs: x_core [128, 2*SLAB], w_core [128, TOT*384],
    sel [128, 128], plus metadata."""
    x = np.asarray(x, np.float32)
    sel = np.zeros((128, 128), ml_dtypes.bfloat16)
    for i in range(64):
        sel[i, i] = 1.0        # S_n
        sel[64 + i, 64 + i] = 1.0  # S_t
    in_maps, metas = [], []
    for k in range(8):
        tplans = core_tplans[k]
        fa = k
        fb = 8 if k < 4 else 9
        x_core = np.empty((128, 2 * SLAB), ml_dtypes.bfloat16)
        for s_bit, f in ((0, fa), (1, fb)):
            cs = [f, f + 10, f + 20]
            xc = x[:, cs]                                   # [B, 3, H, W]
            nrm = xc.transpose(0, 2, 3, 1).reshape(B, SLAB)  # (a*64+b)*3+g
            trn = xc.transpose(0, 3, 2, 1).reshape(B, SLAB)  # (b*64+a)*3+g
            x_core[0:64, s_bit * SLAB:(s_bit + 1) * SLAB] = nrm
            x_core[64:128, s_bit * SLAB:(s_bit + 1) * SLAB] = trn

        tot = sum(p["nch"] for p in tplans)
        wall = np.zeros((tot * 128, DIM), ml_dtypes.bfloat16)
        base = 0
        for p in tplans:
            t = p["tid"]
            Wt = np.asarray(W[t], np.float32).astype(ml_dtypes.bfloat16)
            slots = np.fromiter((s for s, _ in p["wrows"]), np.int64,
                                len(p["wrows"]))
            elems = np.fromiter((e for _, e in p["wrows"]), np.int64,
                                len(p["wrows"]))
            wall[base + slots] = Wt[elems]
            base += p["nch"] * 128
        w_flat = np.ascontiguousarray(
            wall.reshape(tot, 128, DIM).transpose(1, 0, 2)
        ).reshape(128, tot * DIM)

        ng = -(-len(tplans) // YGRP)
        in_maps.append({"x_core": x_core, "w_core": w_flat, "sel": sel})
        metas.append({"tot": tot, "ng": ng})
    return in_maps, metas


# ------------------------------------------------------------- bass program --

def _build_program(tplans, tot, ng):
    from contextlib import ExitStack

    nc = bacc.Bacc("TRN2", target_bir_lowering=False, debug=False,
                   num_devices=1)
    x_d = nc.dram_tensor("x_core", [128, 2 * SLAB], bf16,
                         kind="ExternalInput").ap()
    w_d = nc.dram_tensor("w_core", [128, tot * DIM], bf16,
                         kind="ExternalInput").ap()
    sel_d = nc.dram_tensor("sel", [128, 128], bf16, kind="ExternalInput").ap()
    y_d = nc.dram_tensor("y_core", [ng, 128, YGRP // 2 * DIM], bf16,
                         kind="ExternalOutput").ap()

    # W DMA groups: whole tokens, <= WGROUP chunks each (first group small
    # so compute starts early)
    groups = []          # (tok_lo, tok_hi, chunk0, nchunks)
    lo, c0, acc = 0, 0, 0
    for j, p in enumerate(tplans):
        cap = 10 if not groups else WGROUP
        if acc and acc + p["nch"] > cap:
            groups.append((lo, j, c0, acc))
            lo, c0, acc = j, c0 + acc, 0
        acc += p["nch"]
    groups.append((lo, len(tplans), c0, acc))
    grp_of_tok = {}
    for gi, (tl, th, c0, nchk) in enumerate(groups):
        for j in range(tl, th):
            grp_of_tok[j] = gi

    # flat chunk stream: token pairs (even j -> PE column group 0, odd j ->
    # group 1) have their chunk streams interleaved so adjacent main matmuls
    # land in different column halves of the PE array and stream
    # concurrently. Transposes run SKEW chunks ahead of their main matmuls
    # so the in-order PE queue never stalls on the psum->sbuf copies; chunk
    # pairs share one [128,128] psum tile to halve the copy count.
    flat = []
    pair_start = {}          # pair index -> position in flat
    for pj in range(0, len(tplans), 2):
        pair_start[pj // 2] = len(flat)
        na = tplans[pj]["nch"]
        nb = tplans[pj + 1]["nch"] if pj + 1 < len(tplans) else 0
        for ck in range(max(na, nb)):
            if ck < na:
                flat.append((pj, ck))
            if ck < nb:
                flat.append((pj + 1, ck))
    n_flat = len(flat)
    n_pairs = (len(tplans) + 1) // 2
    start_of_pair = {v: k for k, v in pair_start.items()}
    SKEW = 4
    PSKEW = 3    # gather (DMA) prefetch depth, in token pairs

    with tile.TileContext(nc) as tc, ExitStack() as ctx:
        x_pool = ctx.enter_context(tc.tile_pool(name="x", bufs=2))
        sel_pool = ctx.enter_context(tc.tile_pool(name="sel", bufs=1))
        w_pool = ctx.enter_context(tc.tile_pool(name="w", bufs=3))
        g_pool = ctx.enter_context(
            tc.tile_pool(name="g", bufs=2 * (PSKEW + 1) + 2))
        glb_pool = ctx.enter_context(tc.tile_pool(name="glb", bufs=4))
        pst_pool = ctx.enter_context(
            tc.tile_pool(name="pst", bufs=4, space="PSUM"))
        psa_pool = ctx.enter_context(
            tc.tile_pool(name="psa", bufs=4, space="PSUM"))
        y_pool = ctx.enter_context(tc.tile_pool(name="y", bufs=2))

        x1 = x_pool.tile([128, SLAB], bf16)
        nc.sync.dma_start(x1[:], x_d[:, 0:SLAB])
        sel_sb = sel_pool.tile([128, 128], bf16)
        nc.sync.dma_start(sel_sb[:], sel_d[:])
        x2 = x_pool.tile([128, SLAB], bf16)

        copy_engines = (
            lambda d, s: nc.gpsimd.tensor_copy(d, s),
            lambda d, s: nc.vector.tensor_copy(d, s),
            lambda d, s: nc.scalar.copy(d, s),
        )
        pcopy_engines = (
            lambda d, s: nc.vector.tensor_copy(d, s),
            lambda d, s: nc.scalar.copy(d, s),
        )
        flip = pflip = yflip = 0
        wg_tiles = {}
        x2_issued = False
        g_tiles = {}
        psa_tiles = {}
        y_tiles = {}
        y_done = {}
        ps_pairs = {}
        glb_pairs = {}
        y_tile = None
        chunk_base = {}
        cb = 0
        for j, p in enumerate(tplans):
            chunk_base[j] = cb
            cb += p["nch"]

        def token_prep(j):
            nonlocal flip, x2_issued
            p = tplans[j]
            gi = grp_of_tok[j]
            if gi not in wg_tiles:
                tl, th, c0, nchk = groups[gi]
                wt = w_pool.tile([128, WGROUP * DIM], bf16)
                nc.sync.dma_start(wt[:, 0:nchk * DIM],
                                  w_d[:, c0 * DIM:(c0 + nchk) * DIM])
                wg_tiles[gi] = (wt, c0)
            if not x2_issued and gi >= 1:
                nc.sync.dma_start(x2[:], x_d[:, SLAB:2 * SLAB])
                x2_issued = True
            g = g_pool.tile([128, GTILE], bf16)
            if j < 2 * (PSKEW + 1) + 2:
                nc.gpsimd.memset(g[:], 0.0)   # NaN-safety for fresh SBUF
            # gather runs via SBUF->SBUF DMA on the ACT HWDGE ring (keeps the
            # compute engines free so the PE array stays at high duty)
            for part, src_col, ln, dst in p["copies"]:
                xsrc = x1 if src_col < SLAB else x2
                sc = src_col % SLAB
                nc.scalar.dma_start(
                    g[part:part + 64, dst:dst + ln],
                    xsrc[part:part + 64, sc:sc + ln])
            g_tiles[j] = g

        def prep_pair(pi):
            if pi < n_pairs:
                token_prep(2 * pi)
                if 2 * pi + 1 < len(tplans):
                    token_prep(2 * pi + 1)

        for pi in range(min(PSKEW, n_pairs)):
            prep_pair(pi)

        for i in range(n_flat + SKEW):
            if i < n_flat:
                if i in start_of_pair:
                    prep_pair(start_of_pair[i] + PSKEW)
                j, ck = flat[i]
                p = tplans[j]
                q, col = i // 2, (i % 2) * 64
                if col == 0:
                    ps_pairs[q] = pst_pool.tile([128, 128], f32,
                                                name="pst")
                nc.tensor.matmul(
                    ps_pairs[q][:, col:col + 64],
                    lhsT=g_tiles[j][:, ck * 128:(ck + 1) * 128],
                    rhs=sel_sb[:, p["orients"][ck] * 64:
                               p["orients"][ck] * 64 + 64],
                    start=True, stop=True)
                if i % 2 == 1 or i == n_flat - 1:
                    w = 128 if i % 2 == 1 else 64
                    glb = glb_pool.tile([128, 128], bf16,
                                        name="glb")
                    pcopy_engines[pflip % 2](glb[:, 0:w], ps_pairs[q][:, 0:w])
                    pflip += 1
                    glb_pairs[q] = glb
                    del ps_pairs[q]
            ii = i - SKEW
            if ii >= 0:
                j, ck = flat[ii]
                p = tplans[j]
                nch = p["nch"]
                half = (j % 2) * 64
                if ck == 0 and half == 0:
                    psa_tiles[j // 2] = psa_pool.tile([128, DIM], f32,
                                                      name="psa")
                psa = psa_tiles[j // 2]
                wt, c0 = wg_tiles[grp_of_tok[j]]
                wcol = (chunk_base[j] - c0 + ck) * DIM
                q, col = ii // 2, (ii % 2) * 64
                nc.tensor.matmul(
                    psa[half:half + 64, :],
                    lhsT=glb_pairs[q][:, col:col + 64],
                    rhs=wt[:, wcol:wcol + DIM],
                    start=(ck == 0), stop=(ck == nch - 1),
                    tile_position=(0, half))
                if ck == nch - 1:
                    grp = j // YGRP
                    if grp not in y_tiles:
                        y_tiles[grp] = y_pool.tile(
                            [128, YGRP // 2 * DIM], bf16, name="y")
                        y_done[grp] = 0
                    ycol = ((j % YGRP) // 2) * DIM
                    pcopy_engines[yflip % 2](
                        y_tiles[grp][half:half + 64, ycol:ycol + DIM],
                        psa[half:half + 64, :])
                    yflip += 1
                    y_done[grp] += 1
                    if y_done[grp] == min(YGRP, len(tplans) - grp * YGRP):
                        nc.sync.dma_start(y_d[grp], y_tiles[grp][:])

    nc.compile()
    return nc


# ------------------------------------------------------------------ runner --

def _run_per_core(ncs, in_maps):
    """Per-device execution of 8 distinct single-core programs."""
    from concurrent.futures import ThreadPoolExecutor

    from concourse import mybir as mb
    from concourse.bass2jax import _bass_exec_p, install_neuronx_cc_hook

    install_neuronx_cc_hook()
    devices = jax.devices()[:8]

    def launch(k):
        nc = ncs[k]
        in_names, out_names, out_avals, zero_outs = [], [], [], []
        for alloc in nc.m.functions[0].allocations:
            if not isinstance(alloc, mb.MemoryLocationSet):
                continue
            name = alloc.memorylocations[0].name
            if alloc.kind == "ExternalInput":
                in_names.append(name)
            elif alloc.kind == "ExternalOutput":
                shape = tuple(alloc.tensor_shape)
                dtype = mb.dt.np(alloc.dtype)
                out_names.append(name)
                out_avals.append(jax.core.ShapedArray(shape, dtype))
                zero_outs.append(np.zeros(shape, dtype))
        n_params = len(in_names)
        donate = tuple(range(n_params, n_params + len(out_names)))

        def _body(*args):
            outs = _bass_exec_p.bind(
                *args,
                out_avals=tuple(out_avals),
                in_names=tuple(in_names + out_names),
                out_names=tuple(out_names),
                lowering_input_output_aliases=(),
                sim_require_finite=True,
                sim_require_nnan=True,
                nc=nc,
            )
            return tuple(outs)

        dev = devices[k]
        extras = {}
        for alloc in nc.m.functions[0].allocations:
            if (isinstance(alloc, mb.MemoryLocationSet)
                    and alloc.kind == "ExternalInput"):
                name = alloc.memorylocations[0].name
                if name not in in_maps[k]:
                    extras[name] = np.full(
                        tuple(alloc.tensor_shape), k, mb.dt.np(alloc.dtype))
        args = [jax.device_put(np.asarray(in_maps[k].get(n, extras.get(n))),
                               dev)
                for n in in_names]
        args += [jax.device_put(z, dev) for z in zero_outs]
        out_arrs = jax.jit(_body, donate_argnums=donate,
                           keep_unused=True)(*args)
        return out_names, out_arrs

    with ThreadPoolExecutor(max_workers=8) as ex:
        futs = [ex.submit(launch, k) for k in range(8)]
        handles = [f.result() for f in futs]
    return [
        {name: np.asarray(arr) for name, arr in zip(names, arrs)}
        for names, arrs in handles
    ]


LAST_RESULTS = None


def kernel(x, W, bias, idx_a, idx_b, idx_c, lens):
    global LAST_RESULTS
    x = np.asarray(x, np.float32)
    W = np.asarray(W, np.float32)
    bias = np.asarray(bias, np.float32)
    idx_a = np.asarray(idx_a, np.int32)
    idx_b = np.asarray(idx_b, np.int32)
    idx_c = np.asarray(idx_c, np.int32)
    lens = np.asarray(lens, np.int32)
    assert x.shape == (B, CIN, IMG, IMG) and W.shape == (T, Lmax, DIM)

    if "plan" not in _cache:
        _cache["plan"] = _assign_cores(idx_a, idx_b, idx_c, lens)
    core_tplans, cg = _cache["plan"]
    in_maps, metas = _build_inputs(x, W, bias, core_tplans, cg)
    if "ncs" not in _cache:
        _cache["ncs"] = [
            _build_program(core_tplans[k], metas[k]["tot"], metas[k]["ng"])
            for k in range(8)
        ]
    ncs = _cache["ncs"]

    hook = None
    trace = (os.environ.get("BASS_TRACE")
             and not os.environ.get("BASS_NEVER_TRACE"))
    if trace:
        from antenv.axon_hooks import get_axon_ntff_profile_hook

        hook = get_axon_ntff_profile_hook()
    if hook is not None:
        tmpdir = os.environ.get("KERNEL_TRACE_TMPDIR") or "/tmp/kernel_trace"
        os.makedirs(tmpdir, exist_ok=True)
        with hook(tmpdir, [0]):
            results = _run_per_core(ncs, in_maps)
        LAST_RESULTS = ("ntff", tmpdir, ncs[0])
    else:
        results = _run_per_core(ncs, in_maps)
        LAST_RESULTS = None

    y = np.empty((B, T, DIM), np.float32)
    for k in range(8):
        yk = results[k]["y_core"].astype(np.float32)
        for j, p in enumerate(core_tplans[k]):
            half = (j % 2) * 64
            ycol = ((j % YGRP) // 2) * DIM
            y[:, p["tid"], :] = yk[j // YGRP, half:half + 64,
                                   ycol:ycol + DIM]
    y += bias[None]     # bias applied on host; kernel computes the pure GEMM
    return y


# revision 21
# speedup vs baseline: 1.6454x; 1.6454x over previous
"""Trainium2 Bass kernel for DirectionalFreqEmbed (per-token gather + grouped GEMM).

Token-parallel across 8 NeuronCores. x is kept SBUF-resident in a dual-
orientation layout ([128, 24576] bf16: partitions 0-63 = batches in row-major
spatial order, 64-127 = batches in column-major order; rows interleave the 3
channel-group planes so each token's ragged index set collapses to ~4 long
contiguous runs). The per-token W stream is compacted to the actually-used
l-slots (~5.3 chunks of 128 instead of 12): holes and chunk tails carry zero
W rows so gathered garbage cannot contribute, and the bias is folded in as a
reserved slot-0 row. Each 128-chunk is transposed to [l, batch] with a single
PE matmul against a stacked-identity selector (which also masks the unused
orientation half), then accumulated against the streamed W tile.

kernel(**inputs) takes FULL unsharded inputs and returns the FULL output.
"""
import os
import sys

import ml_dtypes
import numpy as np

for _p in ("/opt/trn_rl_repo", "/root/.axon_site/_ro/trn_rl_repo"):
    if os.path.isdir(_p) and _p not in sys.path:
        sys.path.insert(0, _p)

try:  # the staged antenv lacks axon_hooks; inject a functional stand-in
    import antenv.axon_hooks  # noqa: F401
except ImportError:
    import types as _types

    _hooks = _types.ModuleType("antenv.axon_hooks")
    _hooks._hook = None
    _hooks.get_axon_ntff_profile_hook = lambda: _hooks._hook
    _hooks.set_axon_ntff_profile_hook = lambda h: setattr(_hooks, "_hook", h)
    sys.modules["antenv.axon_hooks"] = _hooks

import jax
import concourse.bass as bass  # noqa: F401
import concourse.tile as tile
from concourse import bacc, mybir

IMG, CIN, DIM, B = 64, 30, 384, 64
T, Lmax = 240, 1452
SLAB = 3 * IMG * IMG          # 12288 interleaved rows per 3-channel slab
MAXCH = 13                    # max 128-chunks per token (Lmax token)
GTILE = MAXCH * 128           # g tile free width
WGROUP = 32                   # max chunks per W DMA group
YGRP = 6                      # tokens per output tile

bf16 = mybir.dt.bfloat16
f32 = mybir.dt.float32

_cache = {}


# ---------------------------------------------------------------- host plan --

def _runs_from_sorted(rows, orig):
    """Maximal +1-contiguous runs over sorted rows; equal rows split runs."""
    runs = []
    i, n = 0, len(rows)
    while i < n:
        j = i
        while j + 1 < n and rows[j + 1] == rows[j] + 1:
            j += 1
        runs.append((int(rows[i]), int(j - i + 1), orig[i:j + 1]))
        i = j + 1
    return runs


def _merge_runs(runs, gap):
    """Merge runs whose src gap is in [0, gap]; holes become zero-W slots."""
    if not runs:
        return []
    runs = sorted(runs, key=lambda r: r[0])
    out = [list(runs[0])]
    for s, ln, orig in runs[1:]:
        g = s - (out[-1][0] + out[-1][1])
        if 0 <= g <= gap:
            out[-1][1] = s + ln - out[-1][0]
            out[-1][2] = np.concatenate([out[-1][2], orig])
        else:
            out.append([s, ln, orig])
    return [(s, ln, orig) for s, ln, orig in out]


def _token_candidate(rn, rt, minrun, gap):
    """Split elements between orientations, build merged runs per orientation."""
    L = len(rn)
    idx = np.arange(L)
    if minrun is None:        # all normal
        o = np.argsort(rn, kind="stable")
        runs_n = _runs_from_sorted(rn[o], o)
        runs_t = []
    elif minrun == 0:         # all transposed
        o = np.argsort(rt, kind="stable")
        runs_t = _runs_from_sorted(rt[o], o)
        runs_n = []
    else:
        o = np.argsort(rn, kind="stable")
        raw = _runs_from_sorted(rn[o], o)
        runs_n, lo = [], []
        for s, ln, orig in raw:
            if ln >= minrun:
                runs_n.append((s, ln, orig))
            else:
                lo.append(orig)
        runs_t = []
        if lo:
            lo = np.concatenate(lo)
            o2 = lo[np.argsort(rt[lo], kind="stable")]
            runs_t = _runs_from_sorted(rt[o2], o2)
    mn = _merge_runs(runs_n, gap)
    mt = _merge_runs(runs_t, gap)
    assert sum(r[1] >= 1 for r in mn + mt) == len(mn + mt)
    assert sum(len(r[2]) for r in mn + mt) == L
    return mn, mt


def _plan_token(t, s_bit, ia, ib, ic, lens):
    """Best gather/pack plan for one token.

    Returns dict: copies [(part, src_col, len, dst_slot)], orients per chunk,
    nch, wrows [(slot, elem)] mapping l-slot -> original W row."""
    L = int(lens[t])
    a = ia[t, :L].astype(np.int64)
    b = ib[t, :L].astype(np.int64)
    g3 = ic[t, :L].astype(np.int64) // 10
    rn = (a * 64 + b) * 3 + g3
    rt = (b * 64 + a) * 3 + g3

    best = None
    for minrun in (None, 0, 24, 48, 96):
        for gap in (16, 48):
            mn, mt = _token_candidate(rn, rt, minrun, gap)
            slots_n = sum(r[1] for r in mn)
            slots_t = sum(r[1] for r in mt)
            nch = sum(-(-s // 128) for s in (slots_n, slots_t) if s)
            ncp = len(mn) + len(mt)
            cost = ncp + 4.0 * nch
            if best is None or cost < best[0]:
                best = (cost, mn, mt, nch)
    _, mn, mt, nch = best

    copies, wrows, orients = [], [], []
    base = s_bit * SLAB
    pos = 0
    for part, merged, rows in ((0, mn, rn), (64, mt, rt)):
        if not merged:
            continue
        pos = -(-pos // 128) * 128    # each section starts on chunk boundary
        sec_start_ch = pos // 128
        for s, ln, orig in merged:
            copies.append((part, base + s, ln, pos))
            r = rows[orig]
            for e, slot in zip(orig, pos + (r - s)):
                wrows.append((int(slot), int(e)))
            pos += ln
        sec_end_ch = -(-pos // 128)
        orients.extend([0 if part == 0 else 1] * (sec_end_ch - sec_start_ch))
        pos = sec_end_ch * 128
    assert len(orients) == nch <= MAXCH
    return {"tid": t, "copies": copies, "orients": orients, "nch": nch,
            "wrows": wrows}


def _assign_cores(ia, ib, ic, lens):
    """Core k gets family k (24 tokens) + a chunk-balanced slice of family
    8 (k<4) or 9 (k>=4). Returns per-core ordered token plans."""
    cg = (np.asarray(ic)[:, 0] % 10).astype(int)
    fam = [list(np.where(cg == f)[0]) for f in range(10)]

    plans = {}
    for t in range(T):
        plans[t] = None  # filled lazily with proper s_bit per core

    def tok_plan(t, s_bit):
        return _plan_token(t, s_bit, ia, ib, ic, lens)

    # chunk counts (s_bit irrelevant for nch)
    nch = {}
    for t in range(T):
        p = tok_plan(t, 0)
        nch[t] = p["nch"]

    cores = []
    for k in range(8):
        cores.append(list(fam[k]))
    for r, ks in ((8, [0, 1, 2, 3]), (9, [4, 5, 6, 7])):
        toks = sorted(fam[r], key=lambda t: -nch[t])
        load = {k: sum(nch[t] for t in cores[k]) for k in ks}
        cnt = {k: 0 for k in ks}
        for t in toks:
            k = min([kk for kk in ks if cnt[kk] < 6], key=lambda kk: load[kk])
            cores[k].append(t)
            load[k] += nch[t]
            cnt[k] += 1

    out = []
    for k in range(8):
        fa = k
        toks = cores[k]
        # pair tokens with similar chunk counts so interleaved main matmuls
        # (PE column-group pairing) rarely run solo; famA stays first (its x
        # slab arrives first)
        fam_a = sorted(toks[:24], key=lambda t: nch[t])
        fam_b = sorted(toks[24:], key=lambda t: nch[t])
        toks = fam_a + fam_b
        tplans = []
        for t in toks:
            s_bit = 0 if cg[t] == fa else 1
            tplans.append(tok_plan(t, s_bit))
        out.append(tplans)
    return out, cg


def _build_inputs(x, W, bias, core_tplans, cg):
    """Per-core numpy inputs: x_core [128, 2*SLAB], w_core [128, TOT*384],
    sel [128, 128], plus metadata."""
    x = np.asarray(x, np.float32)
    sel = np.zeros((128, 128), ml_dtypes.bfloat16)
    for i in range(64):
        sel[i, i] = 1.0        # S_n
        sel[64 + i, 64 + i] = 1.0  # S_t
    in_maps, metas = [], []
    for k in range(8):
        tplans = core_tplans[k]
        fa = k
        fb = 8 if k < 4 else 9
        x_core = np.empty((128, 2 * SLAB), ml_dtypes.bfloat16)
        for s_bit, f in ((0, fa), (1, fb)):
            cs = [f, f + 10, f + 20]
            xc = x[:, cs]                                   # [B, 3, H, W]
            nrm = xc.transpose(0, 2, 3, 1).reshape(B, SLAB)  # (a*64+b)*3+g
            trn = xc.transpose(0, 3, 2, 1).reshape(B, SLAB)  # (b*64+a)*3+g
            x_core[0:64, s_bit * SLAB:(s_bit + 1) * SLAB] = nrm
            x_core[64:128, s_bit * SLAB:(s_bit + 1) * SLAB] = trn

        tot = sum(p["nch"] for p in tplans)
        wall = np.zeros((tot * 128, DIM), ml_dtypes.bfloat16)
        base = 0
        for p in tplans:
            t = p["tid"]
            Wt = np.asarray(W[t], np.float32).astype(ml_dtypes.bfloat16)
            slots = np.fromiter((s for s, _ in p["wrows"]), np.int64,
                                len(p["wrows"]))
            elems = np.fromiter((e for _, e in p["wrows"]), np.int64,
                                len(p["wrows"]))
            wall[base + slots] = Wt[elems]
            base += p["nch"] * 128
        w_flat = np.ascontiguousarray(
            wall.reshape(tot, 128, DIM).transpose(1, 0, 2)
        ).reshape(128, tot * DIM)

        ng = -(-len(tplans) // YGRP)
        in_maps.append({"x_core": x_core, "w_core": w_flat, "sel": sel})
        metas.append({"tot": tot, "ng": ng})
    return in_maps, metas


# ------------------------------------------------------------- bass program --

def _build_program(tplans, tot, ng):
    from contextlib import ExitStack

    nc = bacc.Bacc("TRN2", target_bir_lowering=False, debug=False,
                   num_devices=1)
    x_d = nc.dram_tensor("x_core", [128, 2 * SLAB], bf16,
                         kind="ExternalInput").ap()
    w_d = nc.dram_tensor("w_core", [128, tot * DIM], bf16,
                         kind="ExternalInput").ap()
    sel_d = nc.dram_tensor("sel", [128, 128], bf16, kind="ExternalInput").ap()
    y_d = nc.dram_tensor("y_core", [ng, 128, YGRP // 2 * DIM], bf16,
                         kind="ExternalOutput").ap()

    # W DMA groups: whole tokens, <= WGROUP chunks each (first group small
    # so compute starts early)
    groups = []          # (tok_lo, tok_hi, chunk0, nchunks)
    lo, c0, acc = 0, 0, 0
    for j in range(0, len(tplans), 2):
        pair_n = tplans[j]["nch"] + (
            tplans[j + 1]["nch"] if j + 1 < len(tplans) else 0)
        cap = 12 if not groups else WGROUP
        if acc and acc + pair_n > cap:
            groups.append((lo, j, c0, acc))
            lo, c0, acc = j, c0 + acc, 0
        acc += pair_n
    groups.append((lo, len(tplans), c0, acc))
    assert all(g[3] <= WGROUP for g in groups)
    grp_of_tok = {}
    for gi, (tl, th, c0, nchk) in enumerate(groups):
        for j in range(tl, th):
            grp_of_tok[j] = gi

    # flat chunk stream: token pairs (even j -> PE column group 0, odd j ->
    # group 1) have their chunk streams interleaved so adjacent main matmuls
    # land in different column halves of the PE array and stream
    # concurrently. Transposes run SKEW chunks ahead of their main matmuls
    # so the in-order PE queue never stalls on the psum->sbuf copies; chunk
    # pairs share one [128,128] psum tile to halve the copy count.
    flat = []
    pair_start = {}          # pair index -> position in flat
    for pj in range(0, len(tplans), 2):
        pair_start[pj // 2] = len(flat)
        na = tplans[pj]["nch"]
        nb = tplans[pj + 1]["nch"] if pj + 1 < len(tplans) else 0
        for ck in range(max(na, nb)):
            if ck < na:
                flat.append((pj, ck))
            if ck < nb:
                flat.append((pj + 1, ck))
    n_flat = len(flat)
    n_pairs = (len(tplans) + 1) // 2
    start_of_pair = {v: k for k, v in pair_start.items()}
    SKEW = 4
    PSKEW = 3    # gather (DMA) prefetch depth, in token pairs

    with tile.TileContext(nc) as tc, ExitStack() as ctx:
        x_pool = ctx.enter_context(tc.tile_pool(name="x", bufs=2))
        sel_pool = ctx.enter_context(tc.tile_pool(name="sel", bufs=1))
        w_pool = ctx.enter_context(tc.tile_pool(name="w", bufs=3))
        g_pool = ctx.enter_context(
            tc.tile_pool(name="g", bufs=2 * (PSKEW + 1) + 2))
        glb_pool = ctx.enter_context(tc.tile_pool(name="glb", bufs=4))
        pst_pool = ctx.enter_context(
            tc.tile_pool(name="pst", bufs=4, space="PSUM"))
        psa_pool = ctx.enter_context(
            tc.tile_pool(name="psa", bufs=4, space="PSUM"))
        y_pool = ctx.enter_context(tc.tile_pool(name="y", bufs=2))

        x1 = x_pool.tile([128, SLAB], bf16)
        nc.sync.dma_start(x1[:], x_d[:, 0:SLAB])
        sel_sb = sel_pool.tile([128, 128], bf16)
        nc.sync.dma_start(sel_sb[:], sel_d[:])
        x2 = x_pool.tile([128, SLAB], bf16)

        copy_engines = (
            lambda d, s: nc.gpsimd.tensor_copy(d, s),
            lambda d, s: nc.vector.tensor_copy(d, s),
            lambda d, s: nc.scalar.copy(d, s),
        )
        pcopy_engines = (
            lambda d, s: nc.vector.tensor_copy(d, s),
            lambda d, s: nc.scalar.copy(d, s),
        )
        flip = pflip = yflip = 0
        wg_tiles = {}
        x2_issued = False
        g_tiles = {}
        psa_tiles = {}
        y_tiles = {}
        y_done = {}
        ps_pairs = {}
        glb_pairs = {}
        y_tile = None
        chunk_base = {}
        cb = 0
        for j, p in enumerate(tplans):
            chunk_base[j] = cb
            cb += p["nch"]

        def token_prep(j):
            nonlocal flip, x2_issued
            p = tplans[j]
            gi = grp_of_tok[j]
            if gi not in wg_tiles:
                tl, th, c0, nchk = groups[gi]
                wt = w_pool.tile([128, WGROUP * DIM], bf16)
                nc.sync.dma_start(wt[:, 0:nchk * DIM],
                                  w_d[:, c0 * DIM:(c0 + nchk) * DIM])
                wg_tiles[gi] = (wt, c0)
            if not x2_issued and gi >= 1:
                nc.sync.dma_start(x2[:], x_d[:, SLAB:2 * SLAB])
                x2_issued = True
            g = g_pool.tile([128, GTILE], bf16)
            if j < 2 * (PSKEW + 1) + 2:
                nc.gpsimd.memset(g[:], 0.0)   # NaN-safety for fresh SBUF
            for part, src_col, ln, dst in p["copies"]:
                xsrc = x1 if src_col < SLAB else x2
                sc = src_col % SLAB
                copy_engines[flip % 3](
                    g[part:part + 64, dst:dst + ln],
                    xsrc[part:part + 64, sc:sc + ln])
                flip += 1
            g_tiles[j] = g

        def prep_pair(pi):
            if pi < n_pairs:
                token_prep(2 * pi)
                if 2 * pi + 1 < len(tplans):
                    token_prep(2 * pi + 1)

        for pi in range(min(PSKEW, n_pairs)):
            prep_pair(pi)

        for i in range(n_flat + SKEW):
            if i < n_flat:
                if i in start_of_pair:
                    prep_pair(start_of_pair[i] + PSKEW)
                j, ck = flat[i]
                p = tplans[j]
                q, col = i // 2, (i % 2) * 64
                if col == 0:
                    ps_pairs[q] = pst_pool.tile([128, 128], f32,
                                                name="pst")
                nc.tensor.matmul(
                    ps_pairs[q][:, col:col + 64],
                    lhsT=g_tiles[j][:, ck * 128:(ck + 1) * 128],
                    rhs=sel_sb[:, p["orients"][ck] * 64:
                               p["orients"][ck] * 64 + 64],
                    start=True, stop=True)
                if i % 2 == 1 or i == n_flat - 1:
                    w = 128 if i % 2 == 1 else 64
                    glb = glb_pool.tile([128, 128], bf16,
                                        name="glb")
                    pcopy_engines[pflip % 2](glb[:, 0:w], ps_pairs[q][:, 0:w])
                    pflip += 1
                    glb_pairs[q] = glb
                    del ps_pairs[q]
            ii = i - SKEW
            if ii >= 0:
                j, ck = flat[ii]
                p = tplans[j]
                nch = p["nch"]
                half = (j % 2) * 64
                if ck == 0 and half == 0:
                    psa_tiles[j // 2] = psa_pool.tile([128, DIM], f32,
                                                      name="psa")
                psa = psa_tiles[j // 2]
                wt, c0 = wg_tiles[grp_of_tok[j]]
                wcol = (chunk_base[j] - c0 + ck) * DIM
                q, col = ii // 2, (ii % 2) * 64
                nc.tensor.matmul(
                    psa[half:half + 64, :],
                    lhsT=glb_pairs[q][:, col:col + 64],
                    rhs=wt[:, wcol:wcol + DIM],
                    start=(ck == 0), stop=(ck == nch - 1),
                    tile_position=(0, half))
                if ck == nch - 1:
                    grp = j // YGRP
                    if grp not in y_tiles:
                        y_tiles[grp] = y_pool.tile(
                            [128, YGRP // 2 * DIM], bf16, name="y")
                        y_done[grp] = 0
                    ycol = ((j % YGRP) // 2) * DIM
                    pcopy_engines[yflip % 2](
                        y_tiles[grp][half:half + 64, ycol:ycol + DIM],
                        psa[half:half + 64, :])
                    yflip += 1
                    y_done[grp] += 1
                    if y_done[grp] == min(YGRP, len(tplans) - grp * YGRP):
                        nc.sync.dma_start(y_d[grp], y_tiles[grp][:])

    nc.compile()
    return nc


# ------------------------------------------------------------------ runner --

def _run_per_core(ncs, in_maps):
    """Per-device execution of 8 distinct single-core programs."""
    from concurrent.futures import ThreadPoolExecutor

    from concourse import mybir as mb
    from concourse.bass2jax import _bass_exec_p, install_neuronx_cc_hook

    install_neuronx_cc_hook()
    devices = jax.devices()[:8]

    def launch(k):
        nc = ncs[k]
        in_names, out_names, out_avals, zero_outs = [], [], [], []
        for alloc in nc.m.functions[0].allocations:
            if not isinstance(alloc, mb.MemoryLocationSet):
                continue
            name = alloc.memorylocations[0].name
            if alloc.kind == "ExternalInput":
                in_names.append(name)
            elif alloc.kind == "ExternalOutput":
                shape = tuple(alloc.tensor_shape)
                dtype = mb.dt.np(alloc.dtype)
                out_names.append(name)
                out_avals.append(jax.core.ShapedArray(shape, dtype))
                zero_outs.append(np.zeros(shape, dtype))
        n_params = len(in_names)
        donate = tuple(range(n_params, n_params + len(out_names)))

        def _body(*args):
            outs = _bass_exec_p.bind(
                *args,
                out_avals=tuple(out_avals),
                in_names=tuple(in_names + out_names),
                out_names=tuple(out_names),
                lowering_input_output_aliases=(),
                sim_require_finite=True,
                sim_require_nnan=True,
                nc=nc,
            )
            return tuple(outs)

        dev = devices[k]
        extras = {}
        for alloc in nc.m.functions[0].allocations:
            if (isinstance(alloc, mb.MemoryLocationSet)
                    and alloc.kind == "ExternalInput"):
                name = alloc.memorylocations[0].name
                if name not in in_maps[k]:
                    extras[name] = np.full(
                        tuple(alloc.tensor_shape), k, mb.dt.np(alloc.dtype))
        args = [jax.device_put(np.asarray(in_maps[k].get(n, extras.get(n))),
                               dev)
                for n in in_names]
        args += [jax.device_put(z, dev) for z in zero_outs]
        out_arrs = jax.jit(_body, donate_argnums=donate,
                           keep_unused=True)(*args)
        return out_names, out_arrs

    with ThreadPoolExecutor(max_workers=8) as ex:
        futs = [ex.submit(launch, k) for k in range(8)]
        handles = [f.result() for f in futs]
    return [
        {name: np.asarray(arr) for name, arr in zip(names, arrs)}
        for names, arrs in handles
    ]


LAST_RESULTS = None


def kernel(x, W, bias, idx_a, idx_b, idx_c, lens):
    global LAST_RESULTS
    x = np.asarray(x, np.float32)
    W = np.asarray(W, np.float32)
    bias = np.asarray(bias, np.float32)
    idx_a = np.asarray(idx_a, np.int32)
    idx_b = np.asarray(idx_b, np.int32)
    idx_c = np.asarray(idx_c, np.int32)
    lens = np.asarray(lens, np.int32)
    assert x.shape == (B, CIN, IMG, IMG) and W.shape == (T, Lmax, DIM)

    if "plan" not in _cache:
        _cache["plan"] = _assign_cores(idx_a, idx_b, idx_c, lens)
    core_tplans, cg = _cache["plan"]
    in_maps, metas = _build_inputs(x, W, bias, core_tplans, cg)
    if "ncs" not in _cache:
        _cache["ncs"] = [
            _build_program(core_tplans[k], metas[k]["tot"], metas[k]["ng"])
            for k in range(8)
        ]
    ncs = _cache["ncs"]

    hook = None
    trace = (os.environ.get("BASS_TRACE")
             and not os.environ.get("BASS_NEVER_TRACE"))
    if trace:
        from antenv.axon_hooks import get_axon_ntff_profile_hook

        hook = get_axon_ntff_profile_hook()
    if hook is not None:
        tmpdir = os.environ.get("KERNEL_TRACE_TMPDIR") or "/tmp/kernel_trace"
        os.makedirs(tmpdir, exist_ok=True)
        with hook(tmpdir, [0]):
            results = _run_per_core(ncs, in_maps)
        LAST_RESULTS = ("ntff", tmpdir, ncs[0])
    else:
        results = _run_per_core(ncs, in_maps)
        LAST_RESULTS = None

    y = np.empty((B, T, DIM), np.float32)
    for k in range(8):
        yk = results[k]["y_core"].astype(np.float32)
        for j, p in enumerate(core_tplans[k]):
            half = (j % 2) * 64
            ycol = ((j % YGRP) // 2) * DIM
            y[:, p["tid"], :] = yk[j // YGRP, half:half + 64,
                                   ycol:ycol + DIM]
    y += bias[None]     # bias applied on host; kernel computes the pure GEMM
    return y
